# revision 48
# baseline (speedup 1.0000x reference)
"""DistMult decoder kernel for Trainium2 (8 NeuronCores, SPMD).

Computes rec = (inputs * relation) @ inputs.T for inputs [8192, 512] f32,
relation [512] f32, output [8192, 8192] f32.

Strategy: the output is symmetric (rec[m,n] = sum_k r_k x_mk x_nk), so only
~half the 512x512 blocks are computed on device; the mirror happens during
host-side assembly.  Work is balanced across 8 cores with a rotation trick
that keeps the program SPMD-uniform: core t gets X row-rotated by 2t*512 and
owns global row-blocks (2t, 2t+1), each computing its forward column window
(9 blocks) — 18 block-matmuls per core over a shared 10-column-block xt
window (see the SLOTS comment).  The four slots that are diagonal or
antipodal-duplicate blocks compute only their 128-strip lower triangle
(saving 8.3% of PE rows); the host mirrors diagonals and combines the two
transposed antipodal halves from cores t and t+4 (see TRI).  Matmuls run in
bf16 with fp32 PSUM accumulation; the stationary operand is derived on
device from the resident xt data and the relation vector; outputs are
staged and stored as fp16 and upcast on host.  The host pre-tiles xt into
the exact SBUF layout so all device DMAs are contiguous; loads are chunked
across SWDGE queues and the tensor engine is clock-gate-warmed during the
load window.
"""

import numpy as np
import ml_dtypes

import concourse.bass as bass
import concourse.mybir as mybir
import concourse.tile as tile
from concourse.bass_utils import run_bass_kernel_spmd
from concourse.vector_clock import ScopedClock


# When True, the next TileContext exit emits only the drain chain (no
# all-engine barrier / semaphore clears).  Safe only for the final context
# of the program: the SP drain chain waits on every semaphore, so SP ends
# last and NEFF completion still implies all work (incl. DMA) is done.
_SKIP_TAIL_BARRIER = False


def _split_drain_and_barrier(self, tick_clock, wait_clock):
    """Replacement for TileContext._drain_and_barrier that splits the tail
    drain's semaphore waits across multiple single-wait Drain instructions.
    The walrus build in this environment rejects instructions with more than
    a few sync waits ("Too many sync wait commands"), and the stock tail
    drain waits on every semaphore the kernel used."""
    nc = self.nc
    drain_inst = nc.sync.drain()
    wait_clock.add_sem_waits(
        drain_inst.ins, ScopedClock({None: tick_clock.global_clock})
    )
    si = drain_inst.ins.sync_info
    if si is not None and len(si.on_wait) > 1:
        waits = list(si.on_wait)
        updates = list(si.on_update)
        if _SKIP_TAIL_BARRIER:
            # Final context: the store DMAs (HWDGE) are the dependency
            # leaves -- every matmul/copy/load completion is transitively
            # implied by them (stores wait on copies, copies on matmuls,
            # matmuls on loads; engines retire in order).  Waiting only on
            # the store queues shortens the serial drain chain.
            hw = [w for w in waits if (w.ant_name or "").startswith("DMAHW")]
            if hw:
                waits = hw
        drain_inst.ins.sync_info = mybir.SyncInfo(on_wait=waits[:1], on_update=[])
        for i, w in enumerate(waits[1:]):
            last = i == len(waits) - 2
            d = nc.sync.drain()
            d.ins.sync_info = mybir.SyncInfo(
                on_wait=[w], on_update=updates if last else []
            )
        if len(waits) == 1 and updates:
            d = nc.sync.drain()
            d.ins.sync_info = mybir.SyncInfo(on_wait=[], on_update=updates)

    assert self.sems is not None
    popped = nc._tile_sem_poison_stack.pop()
    assert popped is self._sem_poison
    if _SKIP_TAIL_BARRIER:
        return
    nc.all_engine_barrier()
    nc.clear_and_free_semaphores(list(self.sems.allocated().values()))
    nc.all_engine_barrier()


tile.TileContext._drain_and_barrier = _split_drain_and_barrier

N = 8192            # rows of inputs
D = 512             # feature dim (contraction)
B = 512             # output block size
NB = N // B         # 16 blocks per side
C = 8               # cores
P = 128
KSUB = D // P       # 4 k-subtiles
MSUB = B // P       # 4 m-subtiles per block

# Adjacent-pair triangle scheme: core t owns global row-blocks (2t, 2t+1);
# every row-block r computes its forward window of columns [r, r+8] mod 16,
# which covers each unordered block pair at least once (forward distances
# 0..8 from one side, wrap distances via the other side; antipodal pairs are
# computed twice, last write wins).  The union window of the two adjacent
# rows is only 10 column-blocks, so each core loads 5.25 MB of xt instead of
# the full 8 MB.
#
# (m_block_local, col_window_local) per output slot, ordered by column so
# compute can start as soon as the first xt column-block lands in SBUF.
# m_block_local 0 -> local rows [0, 512) (global 2t); 1 -> [512, 1024)
# (global 2t+1).  Local col j corresponds to global col-block (2t + j).
# Row 0's window: j = 0..8; row 1's window: j = 1..9.
SLOTS = sorted(
    [(0, j) for j in range(9)] + [(1, j) for j in range(1, 10)],
    key=lambda t: (t[1], t[0]),
)
NSLOT = len(SLOTS)  # 18
NBX = 10            # xt column-blocks resident per core

# Triangle slots: the two diagonal blocks (2t,2t), (2t+1,2t+1) are
# symmetric, and the two antipodal blocks (2t,2t+8), (2t+1,2t+9) are each
# computed twice chip-wide (by cores t and t+4, mutually transposed).  For
# these, each m-subtile mi only computes output columns [0, (mi+1)*128) --
# the 128-strip lower triangle -- and the host mirrors the rest (for
# antipodal blocks the partner core supplies the strict upper triangle).
# Lower (not upper) so every partial PSUM write/read region starts at the
# bank base: each region then has a single writer and the tile framework
# emits single-wait copies (the walrus build rejects multi-wait ones).
# Saves 3072 of 8192 PE rows per triangle slot: 147456 -> 135168 rows/core.
TRI = {(0, 0), (1, 1), (0, 8), (1, 9)}

# store-batch boundaries: after slot s, store slots [STORE_AFTER[s], s].
# At most 8 HWDGE DMAs total (shared SP+ACT queue pool): more would force
# queue reuse and a second sync wait, which this walrus build rejects.
# front-loaded batches, single-slot finals: the last store gates the kernel
# tail, so it should be as small as possible


def _mk_store_after(batch_sizes):
    sa = {}
    lo = 0
    for sz in batch_sizes:
        sa[lo + sz - 1] = lo
        lo += sz
    assert lo == NSLOT
    return sa


STORE_AFTER = _mk_store_after([3, 3, 3, 3, 2, 2, 1, 1])   # 8 SP stores

# Loads are split into ~256 KB chunks spread round-robin over the SWDGE
# queues: a single queue only sustains ~70 GB/s, so one big DMA per tensor
# would gate the first matmul ~14 us behind the st load.  With 256 KB
# chunks the first column blocks and weights land in ~4 us.


def _build_nc(repeats: int = 1, **body_kwargs):
    """Build the SPMD program.  repeats>1 runs the whole body that many
    times as sequential TileContexts (used only for timing: the delta
    between repeats isolates device time from dispatch overhead)."""
    nc = bass.Bass()
    # host-pretiled layouts: xt[j, p, o, v] = XT col-block j;
    # rel[p, o] = relation[o*P + p] (weights are derived on device:
    # st = rel * xt[0:2], saving the 1 MB st input).
    xt = nc.declare_dram_parameter(
        "xt", [NBX, P, KSUB, B], mybir.dt.bfloat16, isOutput=False
    )
    rel = nc.declare_dram_parameter(
        "rel", [P, KSUB], mybir.dt.float32, isOutput=False
    )
    # partition-major output: out[p, s*4+mi, v] = block s row (mi*128+p) col v.
    # Makes every store DMA a contiguous per-partition blit of the staging
    # tile; the host untangles the layout during assembly.
    out = nc.declare_dram_parameter(
        "out", [P, NSLOT * MSUB, B], mybir.dt.float16, isOutput=True
    )
    global _SKIP_TAIL_BARRIER
    for rep in range(repeats):
        _SKIP_TAIL_BARRIER = rep == repeats - 1
        _emit_body(nc, xt, rel, out, **body_kwargs)
    _SKIP_TAIL_BARRIER = False
    return nc


def _emit_body(nc, xt, rel, out, do_mm=True, do_copy=True, do_store=True,
               copy_split=True, do_load=True):
    store_after = STORE_AFTER
    with tile.TileContext(nc) as tc:
        with (
            tc.tile_pool(name="xt", bufs=1) as xt_pool,
            tc.tile_pool(name="st", bufs=1) as st_pool,
            tc.tile_pool(name="ob", bufs=1) as out_pool,
            tc.tile_pool(name="ps", bufs=1, space="PSUM") as psum_pool,
        ):
            rel_sb = st_pool.tile([P, KSUB], mybir.dt.float32)
            nc.gpsimd.dma_start(rel_sb[:], rel[:])
            # Tiny observer so DVE sees the rel DMA once; the weight-derive
            # multiplies below then wait only on their xt chunk.
            rel_obs = st_pool.tile([P, KSUB], mybir.dt.float32)
            nc.vector.tensor_copy(rel_obs[:], rel_sb[:])

            # Fully-resident xt chunks (unique dst, no tile reuse -> no
            # extra sync waits).  The first two column blocks load as four
            # 128 KB quarter-chunks each so the first matmuls start ASAP;
            # the rest as 256 KB halves.
            xt_sb = xt_pool.tile([P, NBX, KSUB, B], mybir.dt.bfloat16)
            xt_chunks = {}  # j -> list of o-starts of its chunk DMAs
            for j in range(NBX):
                ostarts = [0, 1, 2, 3] if j < 2 else [0, 2]
                step = 1 if j < 2 else 2
                if do_load or j == 0:
                    for o0 in ostarts:
                        nc.gpsimd.dma_start(
                            xt_sb[:, j, o0 : o0 + step, :],
                            xt[j, :, o0 : o0 + step, :],
                        )
                xt_chunks[j] = ostarts

            # Derive the stationary operand on device: st[p, o, h*B+v] =
            # rel[p, o] * xt[h, p, o, v] for h in {0, 1} (local rows
            # [0, 1024) are exactly xt column blocks 0 and 1).  Each
            # multiply waits on exactly one 128 KB xt quarter-chunk.
            st_sb = st_pool.tile([P, KSUB, 2 * B], mybir.dt.bfloat16)
            # h-outer: matches both chunk-arrival order (block 0's quarters
            # land before block 1's) and consumption order (slot 0 needs all
            # h=0 slices first), so DVE never stalls on a later chunk while
            # an already-resident one waits.
            for h in (0, 1):
                for o in range(KSUB):
                    nc.vector.tensor_tensor(
                        st_sb[:, o, h * B : (h + 1) * B],
                        xt_sb[:, h, o, :],
                        rel_sb[:, o, None].to_broadcast((P, B)),
                        mybir.AluOpType.mult,
                    )

            # statically rotated PSUM banks; unique fp16 staging slot per
            # output tile (no slot reuse -> single-wait copies and stores).
            psum_big = psum_pool.tile([P, 8, B], mybir.dt.float32)
            ob_big = out_pool.tile([P, NSLOT * MSUB, B], mybir.dt.float16)

            if do_mm:
                # PE warm-up: dummy matmuls on zeroed SBUF during the load
                # window flip the HAM clock gate to 8/8 before the real
                # matmuls start (cold PE runs at half clock).  128-row
                # moving operands keep the steady-state PE cost tiny while
                # still giving four activity pulses; they land in bank 0's
                # [0:128) region -- the same shape the w=128 triangle
                # groups write, so the bank's region history stays uniform
                # (single-writer regions -> single-wait copies).
                warm = st_pool.tile([P, 2 * P], mybir.dt.bfloat16)
                nc.vector.memset(warm[:], 0.0)
                for _ in range(4):
                    nc.tensor.matmul(
                        psum_big[:, 0, 0:P],
                        warm[:, 0:P],
                        warm[:, P : 2 * P],
                        start=True,
                        stop=True,
                    )

            if do_copy:
                # Zero-fill the ob columns that triangle slots never copy
                # ([(mi+1)*128, 512) of m-subtiles 0..2), so stores ship
                # finite fp16 there.  One rectangular memset per slot; the
                # later copies overwrite the valid prefix.  On DVE (same
                # engine as those slots' copies) so stores keep a
                # single-engine data dependency.
                for s, (mb, j) in enumerate(SLOTS):
                    if (mb, j) in TRI:
                        for mi in range(MSUB - 1):
                            nc.vector.memset(
                                ob_big[:, s * MSUB + mi, (mi + 1) * P : B],
                                0.0,
                            )

            # Copies PSUM->SBUF split between DVE and ACT per store batch
            # (whole batches on one engine so each store DMA still has a
            # single-engine data dependency = one sync wait).  Batches 2-3
            # (slots 6-11, no triangle slots -- those must share DVE with
            # the garbage-column memsets above) go to ACT, whose copies are
            # ~2x slower.
            eng_of_slot = {}
            bi = 0
            for si in range(NSLOT):
                eng_of_slot[si] = "act" if bi in (2, 3) else "dve"
                if si in store_after:
                    bi += 1

            fg = 0          # full-slot group counter (banks 4-7)
            seen_j = set()
            for s, (mb, j) in enumerate(SLOTS):
                if j not in seen_j:
                    # Dummy weight loads: make PE observe each of xt block
                    # j's chunk DMAs here (Ldweights takes one sync wait
                    # each), so the following matmuls only carry the
                    # PSUM-reuse wait.  HW allows one wait per instruction.
                    # Blocks 0 and 1 are first used by PSUM groups g < 8,
                    # which have no PSUM-reuse wait yet -- their matmuls can
                    # absorb the chunk waits directly (one chunk per k), so
                    # no dummies are needed there.  For later blocks only
                    # the first half-chunk needs an observer: the group's
                    # k=0 matmul carries the PSUM-reuse wait, but its k=2
                    # matmul is free to absorb the second half-chunk's wait.
                    if j >= 2:
                        nc.tensor.ldweights(xt_sb[:, j, xt_chunks[j][0], 0:P])
                    seen_j.add(j)
                tri = (mb, j) in TRI
                for mi in range(MSUB):
                    # triangle slots: m-subtile mi only needs columns
                    # [0, (mi+1)*128) -- the host mirrors the rest.
                    # PSUM banks are split by role so every bank sees a
                    # constant-width write/read history (uniform regions ->
                    # single-wait copies): triangle groups own banks 0-3
                    # (bank = mi, width (mi+1)*128), full groups rotate
                    # over banks 4-7.
                    w = (mi + 1) * P if tri else B
                    if tri:
                        bank = mi
                    else:
                        bank = 4 + fg % 4
                        fg += 1
                    ps = psum_big[:, bank, :]
                    m0 = mb * B + mi * P
                    g = s * MSUB + mi
                    if do_mm:
                        for k in range(KSUB):
                            nc.tensor.matmul(
                                ps[:, 0:w],
                                st_sb[:, k, m0 : m0 + P],
                                xt_sb[:, j, k, 0:w],
                                start=(k == 0),
                                stop=(k == KSUB - 1),
                            )
                    if do_copy:
                        # triangle groups copy only their written columns
                        # (single-writer region -> single sync wait)
                        if copy_split and eng_of_slot[s] == "act":
                            nc.scalar.copy(ob_big[:, g, 0:w], ps[:, 0:w])
                        else:
                            nc.vector.tensor_copy(
                                ob_big[:, g, 0:w], ps[:, 0:w]
                            )
                # Batched stores: at most 8 output DMAs total (one per HWDGE
                # queue) so no DMA ever needs both a data wait and a
                # queue-reuse wait -- instructions only support 1 sync wait.
                if do_store and s in store_after:
                    lo = store_after[s]
                    nc.sync.dma_start(
                        out[:, lo * MSUB : (s + 1) * MSUB, :],
                        ob_big[:, lo * MSUB : (s + 1) * MSUB, :],
                    )


def _make_in_maps(inputs: np.ndarray, relation: np.ndarray):
    xb = inputs.astype(ml_dtypes.bfloat16)
    # rel[p, o] = relation[o*P + p]
    rel_pd = np.ascontiguousarray(
        relation.astype(np.float32).reshape(KSUB, P).T
    )
    in_maps = []
    for t in range(C):
        # core t owns global row-blocks (2t, 2t+1); local index l maps to
        # global row (2t*B + l) % N
        xr = np.roll(xb, -2 * t * B, axis=0)      # [8192, 512]
        # xt[j, p, o, v] = xr[j*B + v, o*P + p], j = 0..9
        xt_c = np.ascontiguousarray(
            xr[: NBX * B].reshape(NBX, B, KSUB, P).transpose(0, 3, 2, 1)
        )
        in_maps.append({"xt": xt_c, "rel": rel_pd})
    return in_maps


def _assemble(outs: list) -> np.ndarray:
    rec = np.empty((N, N), dtype=np.float32)
    # 128-strip lower-triangle mask: valid region of triangle-slot blocks
    strip = np.arange(B) // P
    TRIL = strip[None, :] <= strip[:, None]          # [m, n] computed
    for t in range(C):
        # [128, 72, 512] partition-major -> [18, 512, 512] blocks
        blocks = np.ascontiguousarray(
            np.asarray(outs[t], dtype=np.float32)
            .reshape(P, NSLOT, MSUB, B)
            .transpose(1, 2, 0, 3)
            .reshape(NSLOT, B, B)
        )
        for s, (mb, j) in enumerate(SLOTS):
            r = (2 * t + mb) % NB
            q = (2 * t + j) % NB
            blk = blocks[s]
            dst = rec[r * B : (r + 1) * B, q * B : (q + 1) * B]
            mir = rec[q * B : (q + 1) * B, r * B : (r + 1) * B]
            if (mb, j) in TRI:
                # diagonal blocks (q == r): tril from blk, rest mirrored.
                # antipodal blocks: this core supplies the tril of (r, q)
                # and (transposed) the triu of (q, r); core t+4 supplies
                # the complement.  The masked unions fill both.
                dst[TRIL] = blk[TRIL]
                mir[TRIL.T] = blk.T[TRIL.T]
            else:
                dst[:] = blk
                if q != r:
                    mir[:] = blk.T
    return rec


def kernel(inputs: np.ndarray, relation: np.ndarray) -> np.ndarray:
    nc = _build_nc()
    res = run_bass_kernel_spmd(nc, _make_in_maps(inputs, relation), list(range(C)))
    return _assemble([r["out"] for r in res.results])

